# revision 37
# baseline (speedup 1.0000x reference)
"""Trainium2 Bass kernel for a 4x2048x768 no-scale no-mask attention block.

Sharding: 8 cores = 4 batches x 2 query-halves. Each core computes the
projections for its batch (K/V over the full 2048-key sequence), attention for
its 1024 queries, and the output projection. The program is SPMD-identical
across cores: the host rolls each core's copy of x along the sequence axis so
that the core's own queries always occupy columns 0:1024 — softmax attention
is invariant to a permutation of the keys, so rolling K/V is harmless.

Weight preprocessing on the host (exact algebra, weights only):
  scores  S[i,j] = (x_i Wq^T + bq)(x_j Wk^T + bk)^T
                 = x_i A x_j^T + w[j] + u[i] + c      with A = Wq^T Wk,
                   w = x (Wk^T bq),  u = x (Wq^T bk),  c = bq.bk
  u[i] and c are constant along the softmax axis j and cancel after
  normalization. The kernel computes H = x A^T (one k-style projection) and
  S^T tiles [keys, queries] = HT x xT directly — the q-projection disappears.
  The value path and the out-projection fuse into ONE projection:
  out = P/Z @ (x Wv^T) Wo^T + (bo + Wo bv) = (1/Z) * P @ (x Wvo^T) + boe
  with Wvo = Wo Wv (softmax rows sum to 1 folds bv into boe). w rides the
  vo-projection as one extra moving column, landing token-major as the
  per-partition bias of the exp activation.

Mixed precision (validated ~4e-3 rel err vs 2e-2 budget): the score path
(x, A, ht) is fp16 — 3 extra mantissa bits over bf16 keep softmax logit noise
small; the value path (wvo, vo) is fp16 and exp(S) is bf16 (needs fp32-range
exponents). 16-bit stationaries enable Fast Weight Load so LDWEIGHTS hides
under the matmul streams (fp32 stationaries disable FWL and pace the PE).
fp16 x/weights/output also halve HBM traffic, shrinking the DMA-bound head.

Attention uses exp(S^T) tiles as the STATIONARY operand (4 query-slices of
128), each reused across three moving operands: vo columns 0:512, vo columns
512:768, and a ones column that accumulates Z. Output lands query-major
[q, d] so the 1/Z softmax scale is a per-partition tensor_scalar and the
result DMAs straight to the row-major output. PSUM: 3x [128,1024] out accums
+ S^T staging + Z = exactly 8 banks.
"""

import sys

if "/opt/trn_rl_repo" not in sys.path:
    sys.path.insert(0, "/opt/trn_rl_repo")

import numpy as np

B = 4
S = 2048
D = 768
DT = D // 128  # 6 feature tiles
QH = 1024  # queries per core
NCORES = 8
NJ = S // 128  # 16 key tiles

_CACHE = {}
last_results = None  # BassKernelResults of the most recent run (for test harness)


def _build_nc():
    if "nc" in _CACHE:
        return _CACHE["nc"]

    from concourse import bacc, mybir
    import concourse.tile as tile

    f32 = mybir.dt.float32
    f32r = mybir.dt.float32r
    f16 = mybir.dt.float16
    bf16 = mybir.dt.bfloat16
    AF = mybir.ActivationFunctionType

    nc = bacc.Bacc("TRN2", target_bir_lowering=False, debug=False)

    def dram(name, shape, kind, dt=f32):
        return nc.dram_tensor(name, list(shape), dt, kind=kind).ap()

    xT = dram("xT", (DT, 128, S), "ExternalInput", f16)  # x[b].T rolled, d-tiled
    waT = dram("waT", (DT, 128, D), "ExternalInput", f16)  # (x A^T)-style tiles
    wvoT = dram("wvoT", (DT, 128, D + 1), "ExternalInput", f16)  # [WvoT | Wk^T bq]
    boe = dram("boe", (1, D), "ExternalInput", f16)  # bo + wo @ bv
    out = dram("out", (QH, D), "ExternalOutput", f16)

    with tile.TileContext(nc) as tc:
        # ---- long-lived constants and small state (left side) ----
        consts = tc.alloc_tile_pool(name="consts", bufs=1, side="left")
        ones_f = consts.tile([128, 1], f32, tag="ones_f", name="ones_f")
        nc.vector.memset(ones_f, 1.0)
        onesc = consts.tile([128, 1], f16, tag="onesc", name="onesc")
        nc.vector.tensor_copy(onesc, ones_f)
        onesr_f = consts.tile([1, 128], f32, tag="onesr_f", name="onesr_f")
        nc.vector.memset(onesr_f, 1.0)
        onesr = consts.tile([1, 128], f16, tag="onesr", name="onesr")
        nc.vector.tensor_copy(onesr, onesr_f)
        boe_sb = consts.tile([1, D], f16, tag="boe", name="boe_sb")
        boe_bc = consts.tile([128, D], f16, tag="boe_bc", name="boe_bc")
        wcol = consts.tile([128, NJ], f32, tag="wcol", name="wcol")

        # ---- phase inputs (right side) ----
        xpool = tc.alloc_tile_pool(name="xpool", bufs=1, side="right")
        wapool = tc.alloc_tile_pool(name="wapool", bufs=1, side="right")
        wvopool = tc.alloc_tile_pool(name="wvopool", bufs=1, side="right")

        xt = [
            xpool.tile([128, S], f16, tag=f"xt{d}", name=f"xt{d}") for d in range(DT)
        ]
        wa = [
            wapool.tile([128, D], f16, tag=f"wa{d}", name=f"wa{d}") for d in range(DT)
        ]
        wvo = [
            wvopool.tile([128, D + 1], f16, tag=f"wvo{d}", name=f"wvo{d}")
            for d in range(DT)
        ]

        # DMA issue is serialized per DGE queue (~0.7us per dma_start), so
        # spread the input streams over two queues in consumption order:
        # scalar gets the six jc0 x chunks the HT sweep reads first; sync
        # interleaves the wa tiles with one big per-d transfer for the rest
        # of x (3KB/partition lines), then boe and the vo weights. The
        # gpsimd SWDGE queue stays empty — its first dma only fires ~16us in.
        nc.scalar.dma_start(out=boe_sb, in_=boe)
        for d in range(DT):
            nc.scalar.dma_start(out=xt[d][:, 0:512], in_=xT[d][:, 0:512])
        for d in (1, 3, 5):
            nc.scalar.dma_start(out=xt[d][:, 512:S], in_=xT[d][:, 512:S])
        # wa[0] lands in two pieces so the very first LDWEIGHTS (which only
        # reads columns 0:128) isn't gated on the full 192KB tile. wa tiles
        # pace the jc0 sweep (one every ~1.3us) so they stay ahead of the
        # big jc1-3 x transfers, which aren't consumed until ~16us in.
        nc.sync.dma_start(out=wa[0][:, 0:128], in_=waT[0][:, 0:128])
        nc.sync.dma_start(out=wa[0][:, 128:D], in_=waT[0][:, 128:D])
        sync_order = [("wa", 1), ("wa", 2), ("bx", 0), ("wa", 3), ("bx", 2),
                      ("wa", 4), ("bx", 4), ("wa", 5)]
        for kind, d in sync_order:
            if kind == "wa":
                nc.sync.dma_start(out=wa[d], in_=waT[d])
            else:
                nc.sync.dma_start(out=xt[d][:, 512:S], in_=xT[d][:, 512:S])
        for d in range(DT):
            nc.sync.dma_start(out=wvo[d], in_=wvoT[d])

        # ---- P1: HT[h, j] = (x A^T)^T over the full (rolled) sequence ----
        hpool = tc.alloc_tile_pool(name="hpool", bufs=1, side="left")
        ht = [
            hpool.tile([128, S], f16, tag=f"ht{h}", name=f"ht{h}") for h in range(DT)
        ]
        # stps lives for the whole kernel so the first S^T matmuls don't
        # inherit a write-after-read hazard from the released paB bank.
        stps = tc.alloc_tile_pool(name="stps", bufs=1, space="PSUM")

        # Broadcast boe across partitions through the stps bank BEFORE the
        # HT sweep: done in a released-paA bank instead, these rank-1
        # matmuls inherit the last ht-copy reads and stall the PE (and the
        # whole DVE chain behind boe_bc) at the HT->VO boundary.
        for lo, w in ((0, 512), (512, 256)):
            bb = stps.tile([128, 512], f32, tag="st", name=f"bb{lo}")
            nc.tensor.matmul(
                bb[:, 0:w], onesr, boe_sb[0:1, lo : lo + w], start=True, stop=True
            )
            nc.vector.tensor_copy(boe_bc[:, lo : lo + w], bb[:, 0:w])

        paA = tc.alloc_tile_pool(name="paA", bufs=7, space="PSUM")
        for jc in range(4):
            hps = [
                paA.tile([128, 512], f32, tag="pa", name=f"hps{jc}_{h}")
                for h in range(DT)
            ]
            for d in range(DT):
                for h in range(DT):
                    nc.tensor.matmul(
                        hps[h],
                        wa[d][:, h * 128 : (h + 1) * 128],
                        xt[d][:, jc * 512 : (jc + 1) * 512],
                        start=(d == 0),
                        stop=(d == DT - 1),
                    )
            for h in range(DT):
                nc.scalar.activation(
                    ht[h][:, jc * 512 : (jc + 1) * 512], hps[h], AF.Copy
                )
        paA.release()

        # ---- P2: vo[s, h] token-major fp16, plus the w bias column; also
        #      broadcast boe across partitions with a rank-1 matmul ----
        paB = tc.alloc_tile_pool(name="paB", bufs=3, space="PSUM")
        # ---- P3 pools (the S^T pipeline is primed inside the VO loop) ----
        expool = tc.alloc_tile_pool(name="expool", bufs=6, side="left")

        def emit_st(ib, j):
            io = ib * 512
            stp = stps.tile([128, 512], f32, tag="st", name=f"st{ib}_{j}")
            for d in range(DT):
                nc.tensor.matmul(
                    stp,
                    ht[d][:, j * 128 : (j + 1) * 128],
                    xt[d][:, io : io + 512],
                    start=(d == 0),
                    stop=(d == DT - 1),
                )
            ex = expool.tile([128, 512], bf16, tag="ex", name=f"ex{ib}_{j}")
            nc.scalar.activation(ex, stp, AF.Exp, bias=wcol[:, j : j + 1])
            return ex

        vpool = tc.alloc_tile_pool(name="vpool", bufs=1, side="left")
        v_all = vpool.tile([128, NJ * D], f16, tag="v", name="v_all")
        pre_ex = []
        for s in range(NJ):
            vps = paB.tile([128, D + 1], f32, tag="pb", name=f"vps{s}")
            for d in range(DT):
                nc.tensor.matmul(
                    vps[:, 0:512],
                    xt[d][:, s * 128 : (s + 1) * 128],
                    wvo[d][:, 0:512],
                    start=(d == 0),
                    stop=(d == DT - 1),
                )
                nc.tensor.matmul(
                    vps[:, 512 : D + 1],
                    xt[d][:, s * 128 : (s + 1) * 128],
                    wvo[d][:, 512 : D + 1],
                    start=(d == 0),
                    stop=(d == DT - 1),
                )
            nc.vector.tensor_copy(v_all[:, s * D : (s + 1) * D], vps[:, 0:D])
            nc.vector.tensor_copy(wcol[:, s : s + 1], vps[:, D : D + 1])
            if s >= NJ - 2:
                # Slot block 0's first S^T groups between the last VO tiles:
                # the PE then has queued work to bridge the write-after-read
                # window when the attention psums recycle the VO banks.
                pre_ex.append(emit_st(0, s - (NJ - 2)))
        paB.release()
        pvps = tc.alloc_tile_pool(name="pvps", bufs=3, space="PSUM")
        zps = tc.alloc_tile_pool(name="zps", bufs=1, space="PSUM")
        rzpool = tc.alloc_tile_pool(name="rzpool", bufs=2, side="left")
        outpool = tc.alloc_tile_pool(name="outpool", bufs=3, side="left")

        NB = QH // 512
        for ib in range(NB):
            io = ib * 512
            T0 = pvps.tile([128, 1024], f32, tag="pv", name=f"T0_{ib}")
            T1 = pvps.tile([128, 1024], f32, tag="pv", name=f"T1_{ib}")
            T2 = pvps.tile([128, 1024], f32, tag="pv", name=f"T2_{ib}")
            zp = zps.tile([128, 4], f32, tag="z", name=f"zp{ib}")

            exq = list(pre_ex)

            def consume(jd, T0=T0, T1=T1, T2=T2, zp=zp, exq=exq):
                # PSUM start_tensor_calc clears the enclosing 2KB BANK, so a
                # bank hosting several column-interleaved accumulation groups
                # must be started exactly once (first group) and stopped once
                # (last group); co-bank groups land on pending-zero bytes.
                exd = exq.pop(0)
                st = (jd == 0)
                sp = (jd == NJ - 1)
                if sp:
                    # Finish Z first: the reciprocal + output drain then
                    # overlap the remaining eight PV matmuls.
                    for t in range(4):
                        nc.tensor.matmul(
                            zp[:, t : t + 1],
                            exd[:, t * 128 : (t + 1) * 128],
                            onesc,
                            start=False,
                            stop=(t == 3),
                            skip_group_check=True,
                        )
                for t in range(4):
                    exsl = exd[:, t * 128 : (t + 1) * 128]
                    Tq = T0 if t < 2 else T1
                    qo = (t % 2) * 512
                    nc.tensor.matmul(
                        Tq[:, qo : qo + 512],
                        exsl,
                        v_all[:, jd * D : jd * D + 512],
                        start=st,
                        stop=sp,
                    )
                    nc.tensor.matmul(
                        T2[:, t * 256 : (t + 1) * 256],
                        exsl,
                        v_all[:, jd * D + 512 : jd * D + 768],
                        start=st and t in (0, 2),
                        stop=sp and t in (1, 3),
                        skip_group_check=True,
                    )
                    if not sp:
                        nc.tensor.matmul(
                            zp[:, t : t + 1],
                            exsl,
                            onesc,
                            start=st and t == 0,
                            stop=False,
                            skip_group_check=True,
                        )

            lag = 2
            for j in range(len(exq), NJ):
                exq.append(emit_st(ib, j))
                if j >= lag:
                    consume(j - lag)
            # Drain the pipeline, seeding the next block's first S^T groups
            # between the tail consumes: the PV matmuls hide the stp-slot/
            # exp serialization, and the seeded groups then bridge the PE
            # over this block's psum drain.
            pre_ex = []
            for idx, jd in enumerate(range(NJ - lag, NJ)):
                if ib + 1 < NB:
                    pre_ex.append(emit_st(ib + 1, idx))
                consume(jd)

            rz = rzpool.tile([128, 4], f32, tag="rz", name=f"rz{ib}")
            nc.vector.reciprocal(rz, zp)
            last = ib == QH // 512 - 1
            for t in range(4):
                osb = outpool.tile([128, D], f16, tag="ot", name=f"osb{ib}_{t}")
                Tq = T0 if t < 2 else T1
                qo = (t % 2) * 512
                if last:
                    # Exposed tail: split the 1/Z scale onto the (now idle)
                    # scalar engine so it pipelines with the DVE bias adds.
                    nc.scalar.activation(
                        osb[:, 0:512], Tq[:, qo : qo + 512], AF.Copy,
                        scale=rz[:, t : t + 1],
                    )
                    nc.scalar.activation(
                        osb[:, 512:768], T2[:, t * 256 : (t + 1) * 256], AF.Copy,
                        scale=rz[:, t : t + 1],
                    )
                else:
                    nc.vector.tensor_scalar_mul(
                        osb[:, 0:512], Tq[:, qo : qo + 512], rz[:, t : t + 1]
                    )
                    nc.vector.tensor_scalar_mul(
                        osb[:, 512:768], T2[:, t * 256 : (t + 1) * 256],
                        rz[:, t : t + 1],
                    )
                ro = io + t * 128
                nc.vector.tensor_add(osb[:, 0:512], osb[:, 0:512], boe_bc[:, 0:512])
                nc.sync.dma_start(out=out[ro : ro + 128, 0:512], in_=osb[:, 0:512])
                nc.vector.tensor_add(
                    osb[:, 512:768], osb[:, 512:768], boe_bc[:, 512:768]
                )
                nc.sync.dma_start(
                    out=out[ro : ro + 128, 512:768], in_=osb[:, 512:768]
                )

        for p in (outpool, rzpool, zps, pvps, vpool, expool, stps, hpool,
                  wvopool, wapool, xpool, consts):
            p.release()

    nc.compile()
    _CACHE["nc"] = nc
    return nc


def _shard_inputs(x, wq, bq, wk, bk, wv, bv, wo, bo):
    """Build the 8 per-core input maps (host-side layout + weight algebra)."""
    f = np.float32
    f8 = np.float64
    h = np.float16
    x = np.asarray(x, f)
    wq, wk, wv, wo = (np.asarray(a, f8) for a in (wq, wk, wv, wo))
    bq, bk, bv, bo = (np.asarray(a, f8) for a in (bq, bk, bv, bo))

    def wtiles(w, dt):  # weight [out, in] -> [in-tile, 128, out]
        return np.ascontiguousarray(np.asarray(w, f).T).reshape(DT, 128, -1).astype(dt)

    A = (wq.T @ wk).astype(f)  # [d, e]; H = x @ A.T
    wvo = (wo @ wv).astype(f)  # fused value+out projection
    wkbq_col = (wk.T @ bq).astype(f)  # [768] -> w = x @ wkbq
    wvoT = wtiles(wvo, h)  # (DT, 128, D)
    wvoT_aug = np.concatenate(
        [wvoT, wkbq_col.reshape(DT, 128, 1).astype(h)], axis=2
    )  # (DT, 128, D+1)
    shared = {
        "waT": wtiles(A, h),
        "wvoT": np.ascontiguousarray(wvoT_aug),
        "boe": (bo + wo @ bv).astype(h).reshape(1, D),
    }
    in_maps = []
    for c in range(NCORES):
        b, half = c // 2, c % 2
        xb = np.ascontiguousarray(x[b].T)  # [D, S]
        if half:
            xb = np.concatenate([xb[:, QH:], xb[:, :QH]], axis=1)
        m = dict(shared)
        m["xT"] = np.ascontiguousarray(xb).reshape(DT, 128, S).astype(h)
        in_maps.append(m)
    return in_maps


def kernel(x, wq, bq, wk, bk, wv, bv, wo, bo, trace=False, trace_kwargs=None):
    global last_results
    from concourse.bass_utils import run_bass_kernel_spmd

    nc = _build_nc()
    in_maps = _shard_inputs(x, wq, bq, wk, bk, wv, bv, wo, bo)
    res = run_bass_kernel_spmd(
        nc,
        in_maps,
        core_ids=list(range(NCORES)),
        trace=trace,
        **(trace_kwargs or {}),
    )
    last_results = res
    out = np.empty((B, S, D), np.float32)
    for c in range(NCORES):
        b, half = c // 2, c % 2
        out[b, half * QH : (half + 1) * QH, :] = res.results[c]["out"].astype(np.float32)
    return out


# revision 39
# speedup vs baseline: 1.0211x; 1.0211x over previous
"""Trainium2 Bass kernel for a 4x2048x768 no-scale no-mask attention block.

Sharding: 8 cores = 4 batches x 2 query-halves. Each core computes the
projections for its batch (K/V over the full 2048-key sequence), attention for
its 1024 queries, and the output projection. The program is SPMD-identical
across cores: the host rolls each core's copy of x along the sequence axis so
that the core's own queries always occupy columns 0:1024 — softmax attention
is invariant to a permutation of the keys, so rolling K/V is harmless.

Weight preprocessing on the host (exact algebra, weights only):
  scores  S[i,j] = (x_i Wq^T + bq)(x_j Wk^T + bk)^T
                 = x_i A x_j^T + w[j] + u[i] + c      with A = Wq^T Wk,
                   w = x (Wk^T bq),  u = x (Wq^T bk),  c = bq.bk
  u[i] and c are constant along the softmax axis j and cancel after
  normalization. The kernel computes H = x A^T (one k-style projection) and
  S^T tiles [keys, queries] = HT x xT directly — the q-projection disappears.
  The value path and the out-projection fuse into ONE projection:
  out = P/Z @ (x Wv^T) Wo^T + (bo + Wo bv) = (1/Z) * P @ (x Wvo^T) + boe
  with Wvo = Wo Wv (softmax rows sum to 1 folds bv into boe). w rides the
  vo-projection as one extra moving column, landing token-major as the
  per-partition bias of the exp activation.

Mixed precision (validated ~4e-3 rel err vs 2e-2 budget): the score path
(x, A, ht) is fp16 — 3 extra mantissa bits over bf16 keep softmax logit noise
small; the value path (wvo, vo) is fp16 and exp(S) is bf16 (needs fp32-range
exponents). 16-bit stationaries enable Fast Weight Load so LDWEIGHTS hides
under the matmul streams (fp32 stationaries disable FWL and pace the PE).
fp16 x/weights/output also halve HBM traffic, shrinking the DMA-bound head.

Attention uses exp(S^T) tiles as the STATIONARY operand (4 query-slices of
128), each reused across three moving operands: vo columns 0:512, vo columns
512:768, and a ones column that accumulates Z. Output lands query-major
[q, d] so the 1/Z softmax scale is a per-partition tensor_scalar and the
result DMAs straight to the row-major output. PSUM: 3x [128,1024] out accums
+ S^T staging + Z = exactly 8 banks.
"""

import sys

if "/opt/trn_rl_repo" not in sys.path:
    sys.path.insert(0, "/opt/trn_rl_repo")

import numpy as np

B = 4
S = 2048
D = 768
DT = D // 128  # 6 feature tiles
QH = 1024  # queries per core
NCORES = 8
NJ = S // 128  # 16 key tiles

_CACHE = {}
last_results = None  # BassKernelResults of the most recent run (for test harness)


def _build_nc():
    if "nc" in _CACHE:
        return _CACHE["nc"]

    from concourse import bacc, mybir
    import concourse.tile as tile

    f32 = mybir.dt.float32
    f32r = mybir.dt.float32r
    f16 = mybir.dt.float16
    bf16 = mybir.dt.bfloat16
    AF = mybir.ActivationFunctionType

    nc = bacc.Bacc("TRN2", target_bir_lowering=False, debug=False)

    def dram(name, shape, kind, dt=f32):
        return nc.dram_tensor(name, list(shape), dt, kind=kind).ap()

    xT = dram("xT", (DT, 128, S), "ExternalInput", f16)  # x[b].T rolled, d-tiled
    waT = dram("waT", (DT, 128, D), "ExternalInput", f16)  # (x A^T)-style tiles
    wvoT = dram("wvoT", (DT, 128, D + 1), "ExternalInput", f16)  # [WvoT | Wk^T bq]
    boe = dram("boe", (1, D), "ExternalInput", f16)  # bo + wo @ bv
    out = dram("out", (QH, D), "ExternalOutput", f16)

    with tile.TileContext(nc) as tc:
        # ---- long-lived constants and small state (left side) ----
        consts = tc.alloc_tile_pool(name="consts", bufs=1, side="left")
        ones_f = consts.tile([128, 1], f32, tag="ones_f", name="ones_f")
        nc.vector.memset(ones_f, 1.0)
        onesc = consts.tile([128, 1], f16, tag="onesc", name="onesc")
        nc.vector.tensor_copy(onesc, ones_f)
        onesr_f = consts.tile([1, 128], f32, tag="onesr_f", name="onesr_f")
        nc.vector.memset(onesr_f, 1.0)
        onesr = consts.tile([1, 128], f16, tag="onesr", name="onesr")
        nc.vector.tensor_copy(onesr, onesr_f)
        boe_sb = consts.tile([1, D], f16, tag="boe", name="boe_sb")
        boe_bc = consts.tile([128, D], f16, tag="boe_bc", name="boe_bc")
        wcol = consts.tile([128, NJ], f32, tag="wcol", name="wcol")

        # ---- phase inputs (right side) ----
        xpool = tc.alloc_tile_pool(name="xpool", bufs=1, side="right")
        wapool = tc.alloc_tile_pool(name="wapool", bufs=1, side="right")
        wvopool = tc.alloc_tile_pool(name="wvopool", bufs=1, side="right")

        xt = [
            xpool.tile([128, S], f16, tag=f"xt{d}", name=f"xt{d}") for d in range(DT)
        ]
        wa = [
            wapool.tile([128, D], f16, tag=f"wa{d}", name=f"wa{d}") for d in range(DT)
        ]
        wvo = [
            wvopool.tile([128, D + 1], f16, tag=f"wvo{d}", name=f"wvo{d}")
            for d in range(DT)
        ]

        # DMA issue is serialized per DGE queue (~0.7us per dma_start), so
        # spread the input streams over two queues in consumption order:
        # scalar gets the six jc0 x chunks the HT sweep reads first; sync
        # interleaves the wa tiles with one big per-d transfer for the rest
        # of x (3KB/partition lines), then boe and the vo weights. The
        # gpsimd SWDGE queue stays empty — its first dma only fires ~16us in.
        nc.scalar.dma_start(out=boe_sb, in_=boe)
        for d in range(DT):
            nc.scalar.dma_start(out=xt[d][:, 0:512], in_=xT[d][:, 0:512])
        # wa[0] lands in two pieces so the very first LDWEIGHTS (which only
        # reads columns 0:128) isn't gated on the full 192KB tile. wa tiles
        # pace the jc0 sweep (one every ~1.3us) so they stay ahead of the
        # big jc1-3 x transfers, which aren't consumed until ~16us in.
        nc.sync.dma_start(out=wa[0][:, 0:128], in_=waT[0][:, 0:128])
        nc.sync.dma_start(out=wa[0][:, 128:D], in_=waT[0][:, 128:D])
        sync_order = [("wa", 1), ("wa", 2), ("bx", 0), ("wa", 3), ("bx", 1),
                      ("wa", 4), ("bx", 2), ("wa", 5), ("bx", 3), ("bx", 4),
                      ("bx", 5)]
        for kind, d in sync_order:
            if kind == "wa":
                nc.sync.dma_start(out=wa[d], in_=waT[d])
            else:
                nc.sync.dma_start(out=xt[d][:, 512:S], in_=xT[d][:, 512:S])
        for d in range(DT):
            nc.sync.dma_start(out=wvo[d], in_=wvoT[d])

        # ---- P1: HT[h, j] = (x A^T)^T over the full (rolled) sequence ----
        hpool = tc.alloc_tile_pool(name="hpool", bufs=1, side="left")
        ht = [
            hpool.tile([128, S], f16, tag=f"ht{h}", name=f"ht{h}") for h in range(DT)
        ]
        # stps lives for the whole kernel so the first S^T matmuls don't
        # inherit a write-after-read hazard from the released paB bank.
        stps = tc.alloc_tile_pool(name="stps", bufs=1, space="PSUM")

        # Broadcast boe across partitions through the stps bank BEFORE the
        # HT sweep: done in a released-paA bank instead, these rank-1
        # matmuls inherit the last ht-copy reads and stall the PE (and the
        # whole DVE chain behind boe_bc) at the HT->VO boundary.
        for lo, w in ((0, 512), (512, 256)):
            bb = stps.tile([128, 512], f32, tag="st", name=f"bb{lo}")
            nc.tensor.matmul(
                bb[:, 0:w], onesr, boe_sb[0:1, lo : lo + w], start=True, stop=True
            )
            nc.vector.tensor_copy(boe_bc[:, lo : lo + w], bb[:, 0:w])

        paA = tc.alloc_tile_pool(name="paA", bufs=7, space="PSUM")
        for jc in range(4):
            hps = [
                paA.tile([128, 512], f32, tag="pa", name=f"hps{jc}_{h}")
                for h in range(DT)
            ]
            for d in range(DT):
                for h in range(DT):
                    nc.tensor.matmul(
                        hps[h],
                        wa[d][:, h * 128 : (h + 1) * 128],
                        xt[d][:, jc * 512 : (jc + 1) * 512],
                        start=(d == 0),
                        stop=(d == DT - 1),
                    )
            for h in range(DT):
                nc.scalar.activation(
                    ht[h][:, jc * 512 : (jc + 1) * 512], hps[h], AF.Copy
                )
        paA.release()

        # ---- P2: vo[s, h] token-major fp16, plus the w bias column; also
        #      broadcast boe across partitions with a rank-1 matmul ----
        paB = tc.alloc_tile_pool(name="paB", bufs=3, space="PSUM")
        # ---- P3 pools (the S^T pipeline is primed inside the VO loop) ----
        expool = tc.alloc_tile_pool(name="expool", bufs=6, side="left")

        def emit_st(ib, j):
            io = ib * 512
            stp = stps.tile([128, 512], f32, tag="st", name=f"st{ib}_{j}")
            for d in range(DT):
                nc.tensor.matmul(
                    stp,
                    ht[d][:, j * 128 : (j + 1) * 128],
                    xt[d][:, io : io + 512],
                    start=(d == 0),
                    stop=(d == DT - 1),
                )
            ex = expool.tile([128, 512], bf16, tag="ex", name=f"ex{ib}_{j}")
            nc.scalar.activation(ex, stp, AF.Exp, bias=wcol[:, j : j + 1])
            return ex

        vpool = tc.alloc_tile_pool(name="vpool", bufs=1, side="left")
        v_all = vpool.tile([128, NJ * D], f16, tag="v", name="v_all")
        pre_ex = []
        for s in range(NJ):
            vps = paB.tile([128, D + 1], f32, tag="pb", name=f"vps{s}")
            for d in range(DT):
                nc.tensor.matmul(
                    vps[:, 0:512],
                    xt[d][:, s * 128 : (s + 1) * 128],
                    wvo[d][:, 0:512],
                    start=(d == 0),
                    stop=(d == DT - 1),
                )
                nc.tensor.matmul(
                    vps[:, 512 : D + 1],
                    xt[d][:, s * 128 : (s + 1) * 128],
                    wvo[d][:, 512 : D + 1],
                    start=(d == 0),
                    stop=(d == DT - 1),
                )
            nc.vector.tensor_copy(v_all[:, s * D : (s + 1) * D], vps[:, 0:D])
            nc.vector.tensor_copy(wcol[:, s : s + 1], vps[:, D : D + 1])
            if s >= NJ - 2:
                # Slot block 0's first S^T groups between the last VO tiles:
                # the PE then has queued work to bridge the write-after-read
                # window when the attention psums recycle the VO banks.
                pre_ex.append(emit_st(0, s - (NJ - 2)))
        paB.release()
        pvps = tc.alloc_tile_pool(name="pvps", bufs=3, space="PSUM")
        zps = tc.alloc_tile_pool(name="zps", bufs=1, space="PSUM")
        rzpool = tc.alloc_tile_pool(name="rzpool", bufs=2, side="left")
        outpool = tc.alloc_tile_pool(name="outpool", bufs=3, side="left")

        NB = QH // 512
        for ib in range(NB):
            io = ib * 512
            T0 = pvps.tile([128, 1024], f32, tag="pv", name=f"T0_{ib}")
            T1 = pvps.tile([128, 1024], f32, tag="pv", name=f"T1_{ib}")
            T2 = pvps.tile([128, 1024], f32, tag="pv", name=f"T2_{ib}")
            zp = zps.tile([128, 4], f32, tag="z", name=f"zp{ib}")

            exq = list(pre_ex)

            def consume(jd, T0=T0, T1=T1, T2=T2, zp=zp, exq=exq):
                # PSUM start_tensor_calc clears the enclosing 2KB BANK, so a
                # bank hosting several column-interleaved accumulation groups
                # must be started exactly once (first group) and stopped once
                # (last group); co-bank groups land on pending-zero bytes.
                exd = exq.pop(0)
                st = (jd == 0)
                sp = (jd == NJ - 1)
                if sp:
                    # Finish Z first: the reciprocal + output drain then
                    # overlap the remaining eight PV matmuls.
                    for t in range(4):
                        nc.tensor.matmul(
                            zp[:, t : t + 1],
                            exd[:, t * 128 : (t + 1) * 128],
                            onesc,
                            start=False,
                            stop=(t == 3),
                            skip_group_check=True,
                        )
                for t in range(4):
                    exsl = exd[:, t * 128 : (t + 1) * 128]
                    Tq = T0 if t < 2 else T1
                    qo = (t % 2) * 512
                    nc.tensor.matmul(
                        Tq[:, qo : qo + 512],
                        exsl,
                        v_all[:, jd * D : jd * D + 512],
                        start=st,
                        stop=sp,
                    )
                    nc.tensor.matmul(
                        T2[:, t * 256 : (t + 1) * 256],
                        exsl,
                        v_all[:, jd * D + 512 : jd * D + 768],
                        start=st and t in (0, 2),
                        stop=sp and t in (1, 3),
                        skip_group_check=True,
                    )
                    if not sp:
                        nc.tensor.matmul(
                            zp[:, t : t + 1],
                            exsl,
                            onesc,
                            start=st and t == 0,
                            stop=False,
                            skip_group_check=True,
                        )

            lag = 2
            for j in range(len(exq), NJ):
                exq.append(emit_st(ib, j))
                if j >= lag:
                    consume(j - lag)
            # Drain the pipeline, seeding the next block's first S^T groups
            # between the tail consumes: the PV matmuls hide the stp-slot/
            # exp serialization, and the seeded groups then bridge the PE
            # over this block's psum drain.
            pre_ex = []
            for idx, jd in enumerate(range(NJ - lag, NJ)):
                if ib + 1 < NB:
                    pre_ex.append(emit_st(ib + 1, idx))
                consume(jd)

            rz = rzpool.tile([128, 4], f32, tag="rz", name=f"rz{ib}")
            nc.vector.reciprocal(rz, zp)
            last = ib == QH // 512 - 1
            for t in range(4):
                osb = outpool.tile([128, D], f16, tag="ot", name=f"osb{ib}_{t}")
                Tq = T0 if t < 2 else T1
                qo = (t % 2) * 512
                if last:
                    # Exposed tail: split the 1/Z scale onto the (now idle)
                    # scalar engine so it pipelines with the DVE bias adds.
                    nc.scalar.activation(
                        osb[:, 0:512], Tq[:, qo : qo + 512], AF.Copy,
                        scale=rz[:, t : t + 1],
                    )
                    nc.scalar.activation(
                        osb[:, 512:768], T2[:, t * 256 : (t + 1) * 256], AF.Copy,
                        scale=rz[:, t : t + 1],
                    )
                else:
                    nc.vector.tensor_scalar_mul(
                        osb[:, 0:512], Tq[:, qo : qo + 512], rz[:, t : t + 1]
                    )
                    nc.vector.tensor_scalar_mul(
                        osb[:, 512:768], T2[:, t * 256 : (t + 1) * 256],
                        rz[:, t : t + 1],
                    )
                ro = io + t * 128
                nc.vector.tensor_add(osb[:, 0:512], osb[:, 0:512], boe_bc[:, 0:512])
                nc.sync.dma_start(out=out[ro : ro + 128, 0:512], in_=osb[:, 0:512])
                nc.vector.tensor_add(
                    osb[:, 512:768], osb[:, 512:768], boe_bc[:, 512:768]
                )
                nc.sync.dma_start(
                    out=out[ro : ro + 128, 512:768], in_=osb[:, 512:768]
                )

        for p in (outpool, rzpool, zps, pvps, vpool, expool, stps, hpool,
                  wvopool, wapool, xpool, consts):
            p.release()

    nc.compile()
    _CACHE["nc"] = nc
    return nc


def _shard_inputs(x, wq, bq, wk, bk, wv, bv, wo, bo):
    """Build the 8 per-core input maps (host-side layout + weight algebra)."""
    f = np.float32
    f8 = np.float64
    h = np.float16
    x = np.asarray(x, f)
    wq, wk, wv, wo = (np.asarray(a, f8) for a in (wq, wk, wv, wo))
    bq, bk, bv, bo = (np.asarray(a, f8) for a in (bq, bk, bv, bo))

    def wtiles(w, dt):  # weight [out, in] -> [in-tile, 128, out]
        return np.ascontiguousarray(np.asarray(w, f).T).reshape(DT, 128, -1).astype(dt)

    A = (wq.T @ wk).astype(f)  # [d, e]; H = x @ A.T
    wvo = (wo @ wv).astype(f)  # fused value+out projection
    wkbq_col = (wk.T @ bq).astype(f)  # [768] -> w = x @ wkbq
    wvoT = wtiles(wvo, h)  # (DT, 128, D)
    wvoT_aug = np.concatenate(
        [wvoT, wkbq_col.reshape(DT, 128, 1).astype(h)], axis=2
    )  # (DT, 128, D+1)
    shared = {
        "waT": wtiles(A, h),
        "wvoT": np.ascontiguousarray(wvoT_aug),
        "boe": (bo + wo @ bv).astype(h).reshape(1, D),
    }
    in_maps = []
    for c in range(NCORES):
        b, half = c // 2, c % 2
        xb = np.ascontiguousarray(x[b].T)  # [D, S]
        if half:
            xb = np.concatenate([xb[:, QH:], xb[:, :QH]], axis=1)
        m = dict(shared)
        m["xT"] = np.ascontiguousarray(xb).reshape(DT, 128, S).astype(h)
        in_maps.append(m)
    return in_maps


def kernel(x, wq, bq, wk, bk, wv, bv, wo, bo, trace=False, trace_kwargs=None):
    global last_results
    from concourse.bass_utils import run_bass_kernel_spmd

    nc = _build_nc()
    in_maps = _shard_inputs(x, wq, bq, wk, bk, wv, bv, wo, bo)
    res = run_bass_kernel_spmd(
        nc,
        in_maps,
        core_ids=list(range(NCORES)),
        trace=trace,
        **(trace_kwargs or {}),
    )
    last_results = res
    out = np.empty((B, S, D), np.float32)
    for c in range(NCORES):
        b, half = c // 2, c % 2
        out[b, half * QH : (half + 1) * QH, :] = res.results[c]["out"].astype(np.float32)
    return out


# revision 40
# speedup vs baseline: 1.0257x; 1.0044x over previous
"""Trainium2 Bass kernel for a 4x2048x768 no-scale no-mask attention block.

Sharding: 8 cores = 4 batches x 2 query-halves. Each core computes the
projections for its batch (K/V over the full 2048-key sequence), attention for
its 1024 queries, and the output projection. The program is SPMD-identical
across cores: the host rolls each core's copy of x along the sequence axis so
that the core's own queries always occupy columns 0:1024 — softmax attention
is invariant to a permutation of the keys, so rolling K/V is harmless.

Weight preprocessing on the host (exact algebra, weights only):
  scores  S[i,j] = (x_i Wq^T + bq)(x_j Wk^T + bk)^T
                 = x_i A x_j^T + w[j] + u[i] + c      with A = Wq^T Wk,
                   w = x (Wk^T bq),  u = x (Wq^T bk),  c = bq.bk
  u[i] and c are constant along the softmax axis j and cancel after
  normalization. The kernel computes H = x A^T (one k-style projection) and
  S^T tiles [keys, queries] = HT x xT directly — the q-projection disappears.
  The value path and the out-projection fuse into ONE projection:
  out = P/Z @ (x Wv^T) Wo^T + (bo + Wo bv) = (1/Z) * P @ (x Wvo^T) + boe
  with Wvo = Wo Wv (softmax rows sum to 1 folds bv into boe). w rides the
  vo-projection as one extra moving column, landing token-major as the
  per-partition bias of the exp activation.

Mixed precision (validated ~4e-3 rel err vs 2e-2 budget): the score path
(x, A, ht) is fp16 — 3 extra mantissa bits over bf16 keep softmax logit noise
small; the value path (wvo, vo) is fp16 and exp(S) is bf16 (needs fp32-range
exponents). 16-bit stationaries enable Fast Weight Load so LDWEIGHTS hides
under the matmul streams (fp32 stationaries disable FWL and pace the PE).
fp16 x/weights/output also halve HBM traffic, shrinking the DMA-bound head.

Attention uses exp(S^T) tiles as the STATIONARY operand (4 query-slices of
128), each reused across three moving operands: vo columns 0:512, vo columns
512:768, and a ones column that accumulates Z. Output lands query-major
[q, d] so the 1/Z softmax scale is a per-partition tensor_scalar and the
result DMAs straight to the row-major output. PSUM: 3x [128,1024] out accums
+ S^T staging + Z = exactly 8 banks.
"""

import sys

if "/opt/trn_rl_repo" not in sys.path:
    sys.path.insert(0, "/opt/trn_rl_repo")

import numpy as np

B = 4
S = 2048
D = 768
DT = D // 128  # 6 feature tiles
QH = 1024  # queries per core
NCORES = 8
NJ = S // 128  # 16 key tiles

_CACHE = {}
last_results = None  # BassKernelResults of the most recent run (for test harness)


def _build_nc():
    if "nc" in _CACHE:
        return _CACHE["nc"]

    from concourse import bacc, mybir
    import concourse.tile as tile

    f32 = mybir.dt.float32
    f32r = mybir.dt.float32r
    f16 = mybir.dt.float16
    bf16 = mybir.dt.bfloat16
    AF = mybir.ActivationFunctionType

    nc = bacc.Bacc("TRN2", target_bir_lowering=False, debug=False)

    def dram(name, shape, kind, dt=f32):
        return nc.dram_tensor(name, list(shape), dt, kind=kind).ap()

    xT = dram("xT", (DT, 128, S), "ExternalInput", f16)  # x[b].T rolled, d-tiled
    waT = dram("waT", (DT, 128, D), "ExternalInput", f16)  # (x A^T)-style tiles
    wvoT = dram("wvoT", (DT, 128, D + 1), "ExternalInput", f16)  # [WvoT | Wk^T bq]
    boe = dram("boe", (1, D), "ExternalInput", f16)  # bo + wo @ bv
    out = dram("out", (QH, D), "ExternalOutput", f16)

    with tile.TileContext(nc) as tc:
        # ---- long-lived constants and small state (left side) ----
        consts = tc.alloc_tile_pool(name="consts", bufs=1, side="left")
        ones_f = consts.tile([128, 1], f32, tag="ones_f", name="ones_f")
        nc.vector.memset(ones_f, 1.0)
        onesc = consts.tile([128, 1], f16, tag="onesc", name="onesc")
        nc.vector.tensor_copy(onesc, ones_f)
        onesr_f = consts.tile([1, 128], f32, tag="onesr_f", name="onesr_f")
        nc.vector.memset(onesr_f, 1.0)
        onesr = consts.tile([1, 128], f16, tag="onesr", name="onesr")
        nc.vector.tensor_copy(onesr, onesr_f)
        boe_sb = consts.tile([1, D], f16, tag="boe", name="boe_sb")
        boe_bc = consts.tile([128, D], f16, tag="boe_bc", name="boe_bc")
        wcol = consts.tile([128, NJ], f32, tag="wcol", name="wcol")

        # ---- phase inputs (right side) ----
        xpool = tc.alloc_tile_pool(name="xpool", bufs=1, side="right")
        wapool = tc.alloc_tile_pool(name="wapool", bufs=1, side="right")
        wvopool = tc.alloc_tile_pool(name="wvopool", bufs=1, side="right")

        xt = [
            xpool.tile([128, S], f16, tag=f"xt{d}", name=f"xt{d}") for d in range(DT)
        ]
        wa = [
            wapool.tile([128, D], f16, tag=f"wa{d}", name=f"wa{d}") for d in range(DT)
        ]
        wvo = [
            wvopool.tile([128, D + 1], f16, tag=f"wvo{d}", name=f"wvo{d}")
            for d in range(DT)
        ]

        # DMA issue is serialized per DGE queue (~0.7us per dma_start), so
        # spread the input streams over two queues in consumption order:
        # scalar gets the six jc0 x chunks the HT sweep reads first; sync
        # interleaves the wa tiles with one big per-d transfer for the rest
        # of x (3KB/partition lines), then boe and the vo weights. The
        # gpsimd SWDGE queue stays empty — its first dma only fires ~16us in.
        nc.scalar.dma_start(out=boe_sb, in_=boe)
        for d in range(DT):
            nc.scalar.dma_start(out=xt[d][:, 0:512], in_=xT[d][:, 0:512])
        # wa[0] lands in two pieces so the very first LDWEIGHTS (which only
        # reads columns 0:128) isn't gated on the full 192KB tile. wa tiles
        # pace the jc0 sweep (one every ~1.3us) so they stay ahead of the
        # big jc1-3 x transfers, which aren't consumed until ~16us in.
        nc.sync.dma_start(out=wa[0][:, 0:128], in_=waT[0][:, 0:128])
        nc.sync.dma_start(out=wa[0][:, 128:D], in_=waT[0][:, 128:D])
        sync_order = [("wa", 1), ("wa", 2), ("bx", 0), ("wa", 3), ("bx", 1),
                      ("wa", 4), ("bx", 2), ("wa", 5), ("bx", 3), ("bx", 4),
                      ("bx", 5)]
        for kind, d in sync_order:
            if kind == "wa":
                nc.sync.dma_start(out=wa[d], in_=waT[d])
            else:
                nc.sync.dma_start(out=xt[d][:, 512:S], in_=xT[d][:, 512:S])
        for d in range(DT):
            nc.sync.dma_start(out=wvo[d], in_=wvoT[d])

        # ---- P1: HT[h, j] = (x A^T)^T over the full (rolled) sequence ----
        hpool = tc.alloc_tile_pool(name="hpool", bufs=1, side="left")
        ht = [
            hpool.tile([128, S], f16, tag=f"ht{h}", name=f"ht{h}") for h in range(DT)
        ]
        # stps lives for the whole kernel so the first S^T matmuls don't
        # inherit a write-after-read hazard from the released paB bank.
        stps = tc.alloc_tile_pool(name="stps", bufs=1, space="PSUM")

        # Broadcast boe across partitions through the stps bank BEFORE the
        # HT sweep: done in a released-paA bank instead, these rank-1
        # matmuls inherit the last ht-copy reads and stall the PE (and the
        # whole DVE chain behind boe_bc) at the HT->VO boundary.
        for lo, w in ((0, 512), (512, 256)):
            bb = stps.tile([128, 512], f32, tag="st", name=f"bb{lo}")
            nc.tensor.matmul(
                bb[:, 0:w], onesr, boe_sb[0:1, lo : lo + w], start=True, stop=True
            )
            nc.vector.tensor_copy(boe_bc[:, lo : lo + w], bb[:, 0:w])

        # bufs=6: 24 hps tiles rotate 6 slots, so paB's first banks (reused
        # by vps s0) map to jc3's h0/h1 slots — freed ~4us earlier than the
        # h3/h4 slots a 7-way rotation leaves there. Kills most of the
        # HT->VO write-after-read stall; the jc-boundary slot handoff still
        # clears (h0's copy completes before the next jc's first matmul).
        paA = tc.alloc_tile_pool(name="paA", bufs=6, space="PSUM")
        for jc in range(4):
            hps = [
                paA.tile([128, 512], f32, tag="pa", name=f"hps{jc}_{h}")
                for h in range(DT)
            ]
            for d in range(DT):
                for h in range(DT):
                    nc.tensor.matmul(
                        hps[h],
                        wa[d][:, h * 128 : (h + 1) * 128],
                        xt[d][:, jc * 512 : (jc + 1) * 512],
                        start=(d == 0),
                        stop=(d == DT - 1),
                    )
            for h in range(DT):
                nc.scalar.activation(
                    ht[h][:, jc * 512 : (jc + 1) * 512], hps[h], AF.Copy
                )
        paA.release()

        # ---- P2: vo[s, h] token-major fp16, plus the w bias column; also
        #      broadcast boe across partitions with a rank-1 matmul ----
        paB = tc.alloc_tile_pool(name="paB", bufs=3, space="PSUM")
        # ---- P3 pools (the S^T pipeline is primed inside the VO loop) ----
        expool = tc.alloc_tile_pool(name="expool", bufs=6, side="left")

        def emit_st(ib, j):
            io = ib * 512
            stp = stps.tile([128, 512], f32, tag="st", name=f"st{ib}_{j}")
            for d in range(DT):
                nc.tensor.matmul(
                    stp,
                    ht[d][:, j * 128 : (j + 1) * 128],
                    xt[d][:, io : io + 512],
                    start=(d == 0),
                    stop=(d == DT - 1),
                )
            ex = expool.tile([128, 512], bf16, tag="ex", name=f"ex{ib}_{j}")
            nc.scalar.activation(ex, stp, AF.Exp, bias=wcol[:, j : j + 1])
            return ex

        vpool = tc.alloc_tile_pool(name="vpool", bufs=1, side="left")
        v_all = vpool.tile([128, NJ * D], f16, tag="v", name="v_all")
        pre_ex = []
        for s in range(NJ):
            vps = paB.tile([128, D + 1], f32, tag="pb", name=f"vps{s}")
            for d in range(DT):
                nc.tensor.matmul(
                    vps[:, 0:512],
                    xt[d][:, s * 128 : (s + 1) * 128],
                    wvo[d][:, 0:512],
                    start=(d == 0),
                    stop=(d == DT - 1),
                )
                nc.tensor.matmul(
                    vps[:, 512 : D + 1],
                    xt[d][:, s * 128 : (s + 1) * 128],
                    wvo[d][:, 512 : D + 1],
                    start=(d == 0),
                    stop=(d == DT - 1),
                )
            nc.vector.tensor_copy(v_all[:, s * D : (s + 1) * D], vps[:, 0:D])
            nc.vector.tensor_copy(wcol[:, s : s + 1], vps[:, D : D + 1])
            if s >= NJ - 2:
                # Slot block 0's first S^T groups between the last VO tiles:
                # the PE then has queued work to bridge the write-after-read
                # window when the attention psums recycle the VO banks.
                pre_ex.append(emit_st(0, s - (NJ - 2)))
        paB.release()
        pvps = tc.alloc_tile_pool(name="pvps", bufs=3, space="PSUM")
        zps = tc.alloc_tile_pool(name="zps", bufs=1, space="PSUM")
        rzpool = tc.alloc_tile_pool(name="rzpool", bufs=2, side="left")
        outpool = tc.alloc_tile_pool(name="outpool", bufs=3, side="left")

        NB = QH // 512
        for ib in range(NB):
            io = ib * 512
            T0 = pvps.tile([128, 1024], f32, tag="pv", name=f"T0_{ib}")
            T1 = pvps.tile([128, 1024], f32, tag="pv", name=f"T1_{ib}")
            T2 = pvps.tile([128, 1024], f32, tag="pv", name=f"T2_{ib}")
            zp = zps.tile([128, 4], f32, tag="z", name=f"zp{ib}")

            exq = list(pre_ex)

            def consume(jd, T0=T0, T1=T1, T2=T2, zp=zp, exq=exq):
                # PSUM start_tensor_calc clears the enclosing 2KB BANK, so a
                # bank hosting several column-interleaved accumulation groups
                # must be started exactly once (first group) and stopped once
                # (last group); co-bank groups land on pending-zero bytes.
                exd = exq.pop(0)
                st = (jd == 0)
                sp = (jd == NJ - 1)
                if sp:
                    # Finish Z first: the reciprocal + output drain then
                    # overlap the remaining eight PV matmuls.
                    for t in range(4):
                        nc.tensor.matmul(
                            zp[:, t : t + 1],
                            exd[:, t * 128 : (t + 1) * 128],
                            onesc,
                            start=False,
                            stop=(t == 3),
                            skip_group_check=True,
                        )
                for t in range(4):
                    exsl = exd[:, t * 128 : (t + 1) * 128]
                    Tq = T0 if t < 2 else T1
                    qo = (t % 2) * 512
                    nc.tensor.matmul(
                        Tq[:, qo : qo + 512],
                        exsl,
                        v_all[:, jd * D : jd * D + 512],
                        start=st,
                        stop=sp,
                    )
                    nc.tensor.matmul(
                        T2[:, t * 256 : (t + 1) * 256],
                        exsl,
                        v_all[:, jd * D + 512 : jd * D + 768],
                        start=st and t in (0, 2),
                        stop=sp and t in (1, 3),
                        skip_group_check=True,
                    )
                    if not sp:
                        nc.tensor.matmul(
                            zp[:, t : t + 1],
                            exsl,
                            onesc,
                            start=st and t == 0,
                            stop=False,
                            skip_group_check=True,
                        )

            lag = 2
            for j in range(len(exq), NJ):
                exq.append(emit_st(ib, j))
                if j >= lag:
                    consume(j - lag)
            # Drain the pipeline, seeding the next block's first S^T groups
            # between the tail consumes: the PV matmuls hide the stp-slot/
            # exp serialization, and the seeded groups then bridge the PE
            # over this block's psum drain.
            pre_ex = []
            for idx, jd in enumerate(range(NJ - lag, NJ)):
                if ib + 1 < NB:
                    pre_ex.append(emit_st(ib + 1, idx))
                consume(jd)

            rz = rzpool.tile([128, 4], f32, tag="rz", name=f"rz{ib}")
            nc.vector.reciprocal(rz, zp)
            last = ib == QH // 512 - 1
            for t in range(4):
                osb = outpool.tile([128, D], f16, tag="ot", name=f"osb{ib}_{t}")
                Tq = T0 if t < 2 else T1
                qo = (t % 2) * 512
                if last:
                    # Exposed tail: split the 1/Z scale onto the (now idle)
                    # scalar engine so it pipelines with the DVE bias adds.
                    nc.scalar.activation(
                        osb[:, 0:512], Tq[:, qo : qo + 512], AF.Copy,
                        scale=rz[:, t : t + 1],
                    )
                    nc.scalar.activation(
                        osb[:, 512:768], T2[:, t * 256 : (t + 1) * 256], AF.Copy,
                        scale=rz[:, t : t + 1],
                    )
                else:
                    nc.vector.tensor_scalar_mul(
                        osb[:, 0:512], Tq[:, qo : qo + 512], rz[:, t : t + 1]
                    )
                    nc.vector.tensor_scalar_mul(
                        osb[:, 512:768], T2[:, t * 256 : (t + 1) * 256],
                        rz[:, t : t + 1],
                    )
                ro = io + t * 128
                nc.vector.tensor_add(osb[:, 0:512], osb[:, 0:512], boe_bc[:, 0:512])
                nc.sync.dma_start(out=out[ro : ro + 128, 0:512], in_=osb[:, 0:512])
                nc.vector.tensor_add(
                    osb[:, 512:768], osb[:, 512:768], boe_bc[:, 512:768]
                )
                nc.sync.dma_start(
                    out=out[ro : ro + 128, 512:768], in_=osb[:, 512:768]
                )

        for p in (outpool, rzpool, zps, pvps, vpool, expool, stps, hpool,
                  wvopool, wapool, xpool, consts):
            p.release()

    nc.compile()
    _CACHE["nc"] = nc
    return nc


def _shard_inputs(x, wq, bq, wk, bk, wv, bv, wo, bo):
    """Build the 8 per-core input maps (host-side layout + weight algebra)."""
    f = np.float32
    f8 = np.float64
    h = np.float16
    x = np.asarray(x, f)
    wq, wk, wv, wo = (np.asarray(a, f8) for a in (wq, wk, wv, wo))
    bq, bk, bv, bo = (np.asarray(a, f8) for a in (bq, bk, bv, bo))

    def wtiles(w, dt):  # weight [out, in] -> [in-tile, 128, out]
        return np.ascontiguousarray(np.asarray(w, f).T).reshape(DT, 128, -1).astype(dt)

    A = (wq.T @ wk).astype(f)  # [d, e]; H = x @ A.T
    wvo = (wo @ wv).astype(f)  # fused value+out projection
    wkbq_col = (wk.T @ bq).astype(f)  # [768] -> w = x @ wkbq
    wvoT = wtiles(wvo, h)  # (DT, 128, D)
    wvoT_aug = np.concatenate(
        [wvoT, wkbq_col.reshape(DT, 128, 1).astype(h)], axis=2
    )  # (DT, 128, D+1)
    shared = {
        "waT": wtiles(A, h),
        "wvoT": np.ascontiguousarray(wvoT_aug),
        "boe": (bo + wo @ bv).astype(h).reshape(1, D),
    }
    in_maps = []
    for c in range(NCORES):
        b, half = c // 2, c % 2
        xb = np.ascontiguousarray(x[b].T)  # [D, S]
        if half:
            xb = np.concatenate([xb[:, QH:], xb[:, :QH]], axis=1)
        m = dict(shared)
        m["xT"] = np.ascontiguousarray(xb).reshape(DT, 128, S).astype(h)
        in_maps.append(m)
    return in_maps


def kernel(x, wq, bq, wk, bk, wv, bv, wo, bo, trace=False, trace_kwargs=None):
    global last_results
    from concourse.bass_utils import run_bass_kernel_spmd

    nc = _build_nc()
    in_maps = _shard_inputs(x, wq, bq, wk, bk, wv, bv, wo, bo)
    res = run_bass_kernel_spmd(
        nc,
        in_maps,
        core_ids=list(range(NCORES)),
        trace=trace,
        **(trace_kwargs or {}),
    )
    last_results = res
    out = np.empty((B, S, D), np.float32)
    for c in range(NCORES):
        b, half = c // 2, c % 2
        out[b, half * QH : (half + 1) * QH, :] = res.results[c]["out"].astype(np.float32)
    return out


# revision 41
# speedup vs baseline: 1.0363x; 1.0104x over previous
"""Trainium2 Bass kernel for a 4x2048x768 no-scale no-mask attention block.

Sharding: 8 cores = 4 batches x 2 query-halves. Each core computes the
projections for its batch (K/V over the full 2048-key sequence), attention for
its 1024 queries, and the output projection. The program is SPMD-identical
across cores: the host rolls each core's copy of x along the sequence axis so
that the core's own queries always occupy columns 0:1024 — softmax attention
is invariant to a permutation of the keys, so rolling K/V is harmless.

Weight preprocessing on the host (exact algebra, weights only):
  scores  S[i,j] = (x_i Wq^T + bq)(x_j Wk^T + bk)^T
                 = x_i A x_j^T + w[j] + u[i] + c      with A = Wq^T Wk,
                   w = x (Wk^T bq),  u = x (Wq^T bk),  c = bq.bk
  u[i] and c are constant along the softmax axis j and cancel after
  normalization. The kernel computes H = x A^T (one k-style projection) and
  S^T tiles [keys, queries] = HT x xT directly — the q-projection disappears.
  The value path and the out-projection fuse into ONE projection:
  out = P/Z @ (x Wv^T) Wo^T + (bo + Wo bv) = (1/Z) * P @ (x Wvo^T) + boe
  with Wvo = Wo Wv (softmax rows sum to 1 folds bv into boe). w rides the
  vo-projection as one extra moving column, landing token-major as the
  per-partition bias of the exp activation.

Mixed precision (validated ~4e-3 rel err vs 2e-2 budget): the score path
(x, A, ht) is fp16 — 3 extra mantissa bits over bf16 keep softmax logit noise
small; the value path (wvo, vo) is fp16 and exp(S) is bf16 (needs fp32-range
exponents). 16-bit stationaries enable Fast Weight Load so LDWEIGHTS hides
under the matmul streams (fp32 stationaries disable FWL and pace the PE).
fp16 x/weights/output also halve HBM traffic, shrinking the DMA-bound head.

Attention uses exp(S^T) tiles as the STATIONARY operand (4 query-slices of
128), each reused across three moving operands: vo columns 0:512, vo columns
512:768, and a ones column that accumulates Z. Output lands query-major
[q, d] so the 1/Z softmax scale is a per-partition tensor_scalar and the
result DMAs straight to the row-major output. PSUM: 3x [128,1024] out accums
+ S^T staging + Z = exactly 8 banks.
"""

import sys

if "/opt/trn_rl_repo" not in sys.path:
    sys.path.insert(0, "/opt/trn_rl_repo")

import numpy as np

B = 4
S = 2048
D = 768
DT = D // 128  # 6 feature tiles
QH = 1024  # queries per core
NCORES = 8
NJ = S // 128  # 16 key tiles

_CACHE = {}
last_results = None  # BassKernelResults of the most recent run (for test harness)


def _build_nc():
    if "nc" in _CACHE:
        return _CACHE["nc"]

    from concourse import bacc, mybir
    import concourse.tile as tile

    f32 = mybir.dt.float32
    f32r = mybir.dt.float32r
    f16 = mybir.dt.float16
    bf16 = mybir.dt.bfloat16
    AF = mybir.ActivationFunctionType

    nc = bacc.Bacc("TRN2", target_bir_lowering=False, debug=False)

    def dram(name, shape, kind, dt=f32):
        return nc.dram_tensor(name, list(shape), dt, kind=kind).ap()

    xT = dram("xT", (DT, 128, S), "ExternalInput", f16)  # x[b].T rolled, d-tiled
    waT = dram("waT", (DT, 128, D), "ExternalInput", f16)  # (x A^T)-style tiles
    wvoT = dram("wvoT", (DT, 128, D + 1), "ExternalInput", f16)  # [WvoT | Wk^T bq]
    boe = dram("boe", (1, D), "ExternalInput", f16)  # bo + wo @ bv
    out = dram("out", (QH, D), "ExternalOutput", f16)

    with tile.TileContext(nc) as tc:
        # ---- long-lived constants and small state (left side) ----
        consts = tc.alloc_tile_pool(name="consts", bufs=1, side="left")
        ones_f = consts.tile([128, 1], f32, tag="ones_f", name="ones_f")
        nc.vector.memset(ones_f, 1.0)
        onesc = consts.tile([128, 1], f16, tag="onesc", name="onesc")
        nc.vector.tensor_copy(onesc, ones_f)
        onesr_f = consts.tile([1, 128], f32, tag="onesr_f", name="onesr_f")
        nc.vector.memset(onesr_f, 1.0)
        onesr = consts.tile([1, 128], f16, tag="onesr", name="onesr")
        nc.vector.tensor_copy(onesr, onesr_f)
        boe_sb = consts.tile([1, D], f16, tag="boe", name="boe_sb")
        boe_bc = consts.tile([128, D], f16, tag="boe_bc", name="boe_bc")
        wcol = consts.tile([128, NJ], f32, tag="wcol", name="wcol")

        # ---- phase inputs (right side) ----
        xpool = tc.alloc_tile_pool(name="xpool", bufs=1, side="right")
        wapool = tc.alloc_tile_pool(name="wapool", bufs=1, side="right")
        wvopool = tc.alloc_tile_pool(name="wvopool", bufs=1, side="right")

        xt = [
            xpool.tile([128, S], f16, tag=f"xt{d}", name=f"xt{d}") for d in range(DT)
        ]
        wa = [
            wapool.tile([128, D], f16, tag=f"wa{d}", name=f"wa{d}") for d in range(DT)
        ]
        wvo = [
            wvopool.tile([128, D + 1], f16, tag=f"wvo{d}", name=f"wvo{d}")
            for d in range(DT)
        ]

        # DMA issue is serialized per DGE queue (~0.7us per dma_start), so
        # spread the input streams over two queues in consumption order:
        # scalar gets the six jc0 x chunks the HT sweep reads first; sync
        # interleaves the wa tiles with one big per-d transfer for the rest
        # of x (3KB/partition lines), then boe and the vo weights. The
        # gpsimd SWDGE queue stays empty — its first dma only fires ~16us in.
        nc.scalar.dma_start(out=boe_sb, in_=boe)
        for d in range(DT):
            nc.scalar.dma_start(out=xt[d][:, 0:512], in_=xT[d][:, 0:512])
        # wa[0] lands in two pieces so the very first LDWEIGHTS (which only
        # reads columns 0:128) isn't gated on the full 192KB tile. wa tiles
        # pace the jc0 sweep (one every ~1.3us) so they stay ahead of the
        # big jc1-3 x transfers, which aren't consumed until ~16us in.
        nc.sync.dma_start(out=wa[0][:, 0:128], in_=waT[0][:, 0:128])
        nc.sync.dma_start(out=wa[0][:, 128:D], in_=waT[0][:, 128:D])
        sync_order = [("wa", 1), ("wa", 2), ("bx", 0), ("wa", 3), ("bx", 1),
                      ("wa", 4), ("bx", 2), ("wa", 5), ("bx", 3), ("bx", 4),
                      ("bx", 5)]
        for kind, d in sync_order:
            if kind == "wa":
                nc.sync.dma_start(out=wa[d], in_=waT[d])
            else:
                nc.sync.dma_start(out=xt[d][:, 512:S], in_=xT[d][:, 512:S])
        for d in range(DT):
            nc.sync.dma_start(out=wvo[d], in_=wvoT[d])

        # ---- P1: HT[h, j] = (x A^T)^T over the full (rolled) sequence ----
        hpool = tc.alloc_tile_pool(name="hpool", bufs=1, side="left")
        ht = [
            hpool.tile([128, S], f16, tag=f"ht{h}", name=f"ht{h}") for h in range(DT)
        ]
        # stps lives for the whole kernel so the first S^T matmuls don't
        # inherit a write-after-read hazard from the released paB bank.
        stps = tc.alloc_tile_pool(name="stps", bufs=1, space="PSUM")

        # Broadcast boe across partitions through the stps bank BEFORE the
        # HT sweep: done in a released-paA bank instead, these rank-1
        # matmuls inherit the last ht-copy reads and stall the PE (and the
        # whole DVE chain behind boe_bc) at the HT->VO boundary.
        for lo, w in ((0, 512), (512, 256)):
            bb = stps.tile([128, 512], f32, tag="st", name=f"bb{lo}")
            nc.tensor.matmul(
                bb[:, 0:w], onesr, boe_sb[0:1, lo : lo + w], start=True, stop=True
            )
            nc.vector.tensor_copy(boe_bc[:, lo : lo + w], bb[:, 0:w])

        # bufs=6: 24 hps tiles rotate 6 slots, so paB's first banks (reused
        # by vps s0) map to jc3's h0/h1 slots — freed ~4us earlier than the
        # h3/h4 slots a 7-way rotation leaves there. Kills most of the
        # HT->VO write-after-read stall; the jc-boundary slot handoff still
        # clears (h0's copy completes before the next jc's first matmul).
        paA = tc.alloc_tile_pool(name="paA", bufs=6, space="PSUM")
        for jc in range(4):
            hps = [
                paA.tile([128, 512], f32, tag="pa", name=f"hps{jc}_{h}")
                for h in range(DT)
            ]
            for d in range(DT):
                for h in range(DT):
                    nc.tensor.matmul(
                        hps[h],
                        wa[d][:, h * 128 : (h + 1) * 128],
                        xt[d][:, jc * 512 : (jc + 1) * 512],
                        start=(d == 0),
                        stop=(d == DT - 1),
                    )
            for h in range(DT):
                # Last sweep: alternate whole-tile copies between ACT and
                # DVE so the psum banks (which VO's first vps tiles recycle)
                # free ~2x sooner. Whole tiles keep one writer per ht
                # region, and only three DVE ops precede the v_all casts.
                if jc == 3 and h % 2 == 1:
                    nc.vector.tensor_copy(
                        ht[h][:, jc * 512 : (jc + 1) * 512], hps[h]
                    )
                else:
                    nc.scalar.activation(
                        ht[h][:, jc * 512 : (jc + 1) * 512], hps[h], AF.Copy
                    )
        paA.release()

        # ---- P2: vo[s, h] token-major fp16, plus the w bias column; also
        #      broadcast boe across partitions with a rank-1 matmul ----
        paB = tc.alloc_tile_pool(name="paB", bufs=3, space="PSUM")
        # ---- P3 pools (the S^T pipeline is primed inside the VO loop) ----
        expool = tc.alloc_tile_pool(name="expool", bufs=6, side="left")

        def emit_st(ib, j):
            io = ib * 512
            stp = stps.tile([128, 512], f32, tag="st", name=f"st{ib}_{j}")
            for d in range(DT):
                nc.tensor.matmul(
                    stp,
                    ht[d][:, j * 128 : (j + 1) * 128],
                    xt[d][:, io : io + 512],
                    start=(d == 0),
                    stop=(d == DT - 1),
                )
            ex = expool.tile([128, 512], bf16, tag="ex", name=f"ex{ib}_{j}")
            nc.scalar.activation(ex, stp, AF.Exp, bias=wcol[:, j : j + 1])
            return ex

        vpool = tc.alloc_tile_pool(name="vpool", bufs=1, side="left")
        v_all = vpool.tile([128, NJ * D], f16, tag="v", name="v_all")
        pre_ex = []
        for s in range(NJ):
            vps = paB.tile([128, D + 1], f32, tag="pb", name=f"vps{s}")
            for d in range(DT):
                nc.tensor.matmul(
                    vps[:, 0:512],
                    xt[d][:, s * 128 : (s + 1) * 128],
                    wvo[d][:, 0:512],
                    start=(d == 0),
                    stop=(d == DT - 1),
                )
                nc.tensor.matmul(
                    vps[:, 512 : D + 1],
                    xt[d][:, s * 128 : (s + 1) * 128],
                    wvo[d][:, 512 : D + 1],
                    start=(d == 0),
                    stop=(d == DT - 1),
                )
            nc.vector.tensor_copy(v_all[:, s * D : (s + 1) * D], vps[:, 0:D])
            nc.vector.tensor_copy(wcol[:, s : s + 1], vps[:, D : D + 1])
            if s >= NJ - 2:
                # Slot block 0's first S^T groups between the last VO tiles:
                # the PE then has queued work to bridge the write-after-read
                # window when the attention psums recycle the VO banks.
                pre_ex.append(emit_st(0, s - (NJ - 2)))
        paB.release()
        pvps = tc.alloc_tile_pool(name="pvps", bufs=3, space="PSUM")
        zps = tc.alloc_tile_pool(name="zps", bufs=1, space="PSUM")
        rzpool = tc.alloc_tile_pool(name="rzpool", bufs=2, side="left")
        outpool = tc.alloc_tile_pool(name="outpool", bufs=3, side="left")

        NB = QH // 512
        for ib in range(NB):
            io = ib * 512
            T0 = pvps.tile([128, 1024], f32, tag="pv", name=f"T0_{ib}")
            T1 = pvps.tile([128, 1024], f32, tag="pv", name=f"T1_{ib}")
            T2 = pvps.tile([128, 1024], f32, tag="pv", name=f"T2_{ib}")
            zp = zps.tile([128, 4], f32, tag="z", name=f"zp{ib}")

            exq = list(pre_ex)

            def consume(jd, T0=T0, T1=T1, T2=T2, zp=zp, exq=exq):
                # PSUM start_tensor_calc clears the enclosing 2KB BANK, so a
                # bank hosting several column-interleaved accumulation groups
                # must be started exactly once (first group) and stopped once
                # (last group); co-bank groups land on pending-zero bytes.
                exd = exq.pop(0)
                st = (jd == 0)
                sp = (jd == NJ - 1)
                if sp:
                    # Finish Z first: the reciprocal + output drain then
                    # overlap the remaining eight PV matmuls.
                    for t in range(4):
                        nc.tensor.matmul(
                            zp[:, t : t + 1],
                            exd[:, t * 128 : (t + 1) * 128],
                            onesc,
                            start=False,
                            stop=(t == 3),
                            skip_group_check=True,
                        )
                for t in range(4):
                    exsl = exd[:, t * 128 : (t + 1) * 128]
                    Tq = T0 if t < 2 else T1
                    qo = (t % 2) * 512
                    nc.tensor.matmul(
                        Tq[:, qo : qo + 512],
                        exsl,
                        v_all[:, jd * D : jd * D + 512],
                        start=st,
                        stop=sp,
                    )
                    nc.tensor.matmul(
                        T2[:, t * 256 : (t + 1) * 256],
                        exsl,
                        v_all[:, jd * D + 512 : jd * D + 768],
                        start=st and t in (0, 2),
                        stop=sp and t in (1, 3),
                        skip_group_check=True,
                    )
                    if not sp:
                        nc.tensor.matmul(
                            zp[:, t : t + 1],
                            exsl,
                            onesc,
                            start=st and t == 0,
                            stop=False,
                            skip_group_check=True,
                        )

            lag = 2
            for j in range(len(exq), NJ):
                exq.append(emit_st(ib, j))
                if j >= lag:
                    consume(j - lag)
            # Drain the pipeline, seeding the next block's first S^T groups
            # between the tail consumes: the PV matmuls hide the stp-slot/
            # exp serialization, and the seeded groups then bridge the PE
            # over this block's psum drain.
            pre_ex = []
            for idx, jd in enumerate(range(NJ - lag, NJ)):
                if ib + 1 < NB:
                    pre_ex.append(emit_st(ib + 1, idx))
                consume(jd)

            rz = rzpool.tile([128, 4], f32, tag="rz", name=f"rz{ib}")
            nc.vector.reciprocal(rz, zp)
            last = ib == QH // 512 - 1
            for t in range(4):
                osb = outpool.tile([128, D], f16, tag="ot", name=f"osb{ib}_{t}")
                Tq = T0 if t < 2 else T1
                qo = (t % 2) * 512
                if last:
                    # Exposed tail: split the 1/Z scale onto the (now idle)
                    # scalar engine so it pipelines with the DVE bias adds.
                    nc.scalar.activation(
                        osb[:, 0:512], Tq[:, qo : qo + 512], AF.Copy,
                        scale=rz[:, t : t + 1],
                    )
                    nc.scalar.activation(
                        osb[:, 512:768], T2[:, t * 256 : (t + 1) * 256], AF.Copy,
                        scale=rz[:, t : t + 1],
                    )
                else:
                    nc.vector.tensor_scalar_mul(
                        osb[:, 0:512], Tq[:, qo : qo + 512], rz[:, t : t + 1]
                    )
                    nc.vector.tensor_scalar_mul(
                        osb[:, 512:768], T2[:, t * 256 : (t + 1) * 256],
                        rz[:, t : t + 1],
                    )
                ro = io + t * 128
                nc.vector.tensor_add(osb[:, 0:512], osb[:, 0:512], boe_bc[:, 0:512])
                nc.sync.dma_start(out=out[ro : ro + 128, 0:512], in_=osb[:, 0:512])
                nc.vector.tensor_add(
                    osb[:, 512:768], osb[:, 512:768], boe_bc[:, 512:768]
                )
                nc.sync.dma_start(
                    out=out[ro : ro + 128, 512:768], in_=osb[:, 512:768]
                )

        for p in (outpool, rzpool, zps, pvps, vpool, expool, stps, hpool,
                  wvopool, wapool, xpool, consts):
            p.release()

    nc.compile()
    _CACHE["nc"] = nc
    return nc


def _shard_inputs(x, wq, bq, wk, bk, wv, bv, wo, bo):
    """Build the 8 per-core input maps (host-side layout + weight algebra)."""
    f = np.float32
    f8 = np.float64
    h = np.float16
    x = np.asarray(x, f)
    wq, wk, wv, wo = (np.asarray(a, f8) for a in (wq, wk, wv, wo))
    bq, bk, bv, bo = (np.asarray(a, f8) for a in (bq, bk, bv, bo))

    def wtiles(w, dt):  # weight [out, in] -> [in-tile, 128, out]
        return np.ascontiguousarray(np.asarray(w, f).T).reshape(DT, 128, -1).astype(dt)

    A = (wq.T @ wk).astype(f)  # [d, e]; H = x @ A.T
    wvo = (wo @ wv).astype(f)  # fused value+out projection
    wkbq_col = (wk.T @ bq).astype(f)  # [768] -> w = x @ wkbq
    wvoT = wtiles(wvo, h)  # (DT, 128, D)
    wvoT_aug = np.concatenate(
        [wvoT, wkbq_col.reshape(DT, 128, 1).astype(h)], axis=2
    )  # (DT, 128, D+1)
    shared = {
        "waT": wtiles(A, h),
        "wvoT": np.ascontiguousarray(wvoT_aug),
        "boe": (bo + wo @ bv).astype(h).reshape(1, D),
    }
    in_maps = []
    for c in range(NCORES):
        b, half = c // 2, c % 2
        xb = np.ascontiguousarray(x[b].T)  # [D, S]
        if half:
            xb = np.concatenate([xb[:, QH:], xb[:, :QH]], axis=1)
        m = dict(shared)
        m["xT"] = np.ascontiguousarray(xb).reshape(DT, 128, S).astype(h)
        in_maps.append(m)
    return in_maps


def kernel(x, wq, bq, wk, bk, wv, bv, wo, bo, trace=False, trace_kwargs=None):
    global last_results
    from concourse.bass_utils import run_bass_kernel_spmd

    nc = _build_nc()
    in_maps = _shard_inputs(x, wq, bq, wk, bk, wv, bv, wo, bo)
    res = run_bass_kernel_spmd(
        nc,
        in_maps,
        core_ids=list(range(NCORES)),
        trace=trace,
        **(trace_kwargs or {}),
    )
    last_results = res
    out = np.empty((B, S, D), np.float32)
    for c in range(NCORES):
        b, half = c // 2, c % 2
        out[b, half * QH : (half + 1) * QH, :] = res.results[c]["out"].astype(np.float32)
    return out
